# revision 50
# baseline (speedup 1.0000x reference)
"""Causal self-attention (B=2, T=2048, C=2048, 16 heads) on 8 Trainium2 cores.

Sharding: tensor-parallel over heads - 2 heads per core. Each core computes
q/k/v projections for its head group, causal attention, and a partial output
projection (row-parallel Wo); the host sums the 8 partial outputs.

v2 layout notes (per core):
  - Q/K projections run in fp8e4 (e4m3) DoubleRow mode: operands are
    pre-scaled into fp8 range on the host (scales fold out through the
    exp: exp(S_raw * EXP_SCALE) with EXP_SCALE = 2^-40). 2x PE rate.
  - Everything else is bf16 (x, V path, P, attention out, Wo, y partials);
    accumulation stays in f32 PSUM. y partials are written bf16 and summed
    in f32 on the host.
  - Attention works on S^T tiles [k,q] so no transposes are needed:
      S^T tile = kT_chunk.T @ qT_window
      P^T      = exp(S^T * EXP_SCALE)   (ACT, bf16 out; diag tiles masked
                                         by gpsimd affine_select)
      outT    += v_tile.T @ P^T
      rowsum  += ones.T @ P^T           (PE, accumulated in PSUM [1,W])
    Normalization (1/rowsum broadcast-mul) is deferred one q-window.
  - Output projection for a finished q-window is interleaved into the next
    window's attention steps to fill PE gaps while ACT runs exp.
"""

import math
import sys
from contextlib import ExitStack

import numpy as np

sys.path.insert(0, "/opt/trn_rl_repo")

import concourse.bass as bass  # noqa: E402
import concourse.tile as tile  # noqa: E402
from concourse import bacc, mybir  # noqa: E402

F32 = mybir.dt.float32
BF16 = mybir.dt.bfloat16
FP8 = mybir.dt.float8e4

B_FULL, T_FULL, C_FULL = 2, 2048, 2048
N_HEADS, HEAD_DIM = 16, 128
N_CORES = 8
H_LOC = N_HEADS // N_CORES  # 2 heads per core
C_LOC = H_LOC * HEAD_DIM  # 256 output dims per core

WIN = 512  # attention q-window / projection token window
SUPER = 1024  # DMA super-window (2 projection windows per strip)

import os
QK_FP8 = os.environ.get("QK_FP8", "1") == "1"
SX = 32.0  # x fp8 scale
# NOTE: device fp8e4 treats exponent 1111 as inf/nan (max normal ~240),
# unlike ml_dtypes e4m3fn (max 448) - keep all scaled magnitudes < 224.
SQ = 65536.0  # q total scale is SX*SQ (1/sqrt(D) folded into Wq on host)
SK = 8192.0  # k total scale is SX*SK
EXP_SCALE = 1.0 / (SX * SX * SQ * SK) if QK_FP8 else 1.0  # 2^-39


def build_program(Bb=B_FULL, Tt=T_FULL, Cc=C_FULL):
    BT = Bb * Tt
    n_kc = Cc // 128  # 16 contraction chunks
    n_pair = n_kc // 2  # 8 fp8 DoubleRow pairs
    n_sw = BT // SUPER  # 4 DMA super-windows
    n_win = SUPER // WIN  # 2 projection windows per super-window
    n_qw = Tt // WIN  # 4 attention q-windows per batch
    sub = WIN // 128  # 4 token tiles per window
    n_nw = Cc // WIN  # 4 output column windows

    nc = bacc.Bacc("TRN2", target_bir_lowering=False, debug=False,
                   num_devices=N_CORES)

    if QK_FP8:
        x8_ap = nc.dram_tensor("x8", [128, n_pair, 2, BT], FP8,
                               kind="ExternalInput").ap()
    xb_ap = nc.dram_tensor("xb", [128, n_kc, BT], BF16,
                           kind="ExternalInput").ap()
    dbg_mode = int(os.environ.get("DBG_DUMP", "0"))
    dbg = dbg_mode > 0
    if dbg:
        dq_ap = nc.dram_tensor("dq", [128, H_LOC, BT], BF16,
                               kind="ExternalOutput").ap()
        dk_ap = nc.dram_tensor("dk", [128, H_LOC, BT], BF16,
                               kind="ExternalOutput").ap()
        dv_ap = nc.dram_tensor("dv", [128, BT // 128, C_LOC], BF16,
                               kind="ExternalOutput").ap()
        dot_ap = nc.dram_tensor("dot", [128, H_LOC, BT], BF16,
                                kind="ExternalOutput").ap()
    if QK_FP8:
        wq_ap = nc.dram_tensor("wq", [128, n_pair, 2, C_LOC], FP8,
                               kind="ExternalInput").ap()
        wk_ap = nc.dram_tensor("wk", [128, n_pair, 2, C_LOC], FP8,
                               kind="ExternalInput").ap()
    else:
        wq_ap = nc.dram_tensor("wq", [128, n_kc, C_LOC], BF16,
                               kind="ExternalInput").ap()
        wk_ap = nc.dram_tensor("wk", [128, n_kc, C_LOC], BF16,
                               kind="ExternalInput").ap()
    wv_ap = nc.dram_tensor("wv", [128, n_kc, C_LOC], BF16,
                           kind="ExternalInput").ap()
    wo_ap = nc.dram_tensor("wo", [128, H_LOC, Cc], BF16,
                           kind="ExternalInput").ap()
    y_ap = nc.dram_tensor("y", [BT, Cc], BF16, kind="ExternalOutput").ap()

    with tile.TileContext(nc) as tc, ExitStack() as ctx:
        const = ctx.enter_context(tc.tile_pool(name="const", bufs=1))
        wpool = ctx.enter_context(tc.tile_pool(name="wpool", bufs=1))
        big = ctx.enter_context(tc.tile_pool(name="big", bufs=1))

        ones_f32 = const.tile([128, 1], F32, tag="ones_f32")
        nc.any.memset(ones_f32[:], 1.0)
        ones_col = const.tile([128, 1], BF16, tag="ones_col")
        nc.vector.tensor_copy(ones_col[:], ones_f32[:])
        # additive causal mask for the diagonal 128-block of S^T tiles:
        # maskc[p, j] = 0 where j >= p (keep), -BIG where j < p
        maskz = const.tile([128, 128], F32, tag="maskz")
        nc.any.memset(maskz[:], 0.0)
        maskc = const.tile([128, 128], F32, tag="maskc")
        nc.gpsimd.affine_select(
            out=maskc[:], in_=maskz[:],
            compare_op=mybir.AluOpType.is_ge, fill=-3.0e38, base=0,
            pattern=[[1, 128]], channel_multiplier=-1)

        # Persistent SBUF tensors
        if QK_FP8:
            wq_s = wpool.tile([128, n_pair, 2, C_LOC], FP8, tag="wq")
            wk_s = wpool.tile([128, n_pair, 2, C_LOC], FP8, tag="wk")
        else:
            wq_s = wpool.tile([128, n_kc, C_LOC], BF16, tag="wq")
            wk_s = wpool.tile([128, n_kc, C_LOC], BF16, tag="wk")
        wv_s = wpool.tile([128, n_kc, C_LOC], BF16, tag="wv")
        wo_s = wpool.tile([128, H_LOC, Cc], BF16, tag="wo")
        qT_s = big.tile([128, H_LOC, BT], BF16, tag="qT")
        kT_s = big.tile([128, H_LOC, BT], BF16, tag="kT")
        v_s = big.tile([128, BT // 128, C_LOC], BF16, tag="v")
        ot_s = big.tile([128, H_LOC, BT], BF16, tag="ot")

        # ---- Stage 1: q/k/v projections --------------------------------
        with nc.named_scope("qkv_proj"), ExitStack() as s1:
            x8pool = s1.enter_context(tc.tile_pool(name="x8pool", bufs=10))
            xbpool = s1.enter_context(tc.tile_pool(name="xbpool", bufs=20))
            ps_qk = s1.enter_context(
                tc.tile_pool(name="ps_qk", bufs=1, space="PSUM"))
            ps_v = s1.enter_context(
                tc.tile_pool(name="ps_v", bufs=1, space="PSUM"))

            nc.sync.dma_start(wq_s[:], wq_ap)

            for sw in range(n_sw):
                ts0 = sw * SUPER
                x8t = []
                xbt = []
                for pair in range(n_pair):
                    t = x8pool.tile([128, 2, SUPER], FP8, tag="x8",
                                    name=f"x8_{sw}_{pair}")
                    nc.sync.dma_start(t[:], x8_ap[:, pair, :, ts0:ts0 + SUPER])
                    x8t.append(t)
                    if sw == 0 and pair == 0:
                        # first q matmul needs only wq + x8 pair 0; wk can
                        # land while it runs
                        nc.sync.dma_start(wk_s[:], wk_ap)
                if sw == 0:
                    # q/k matmuls only need wq/wk + x8 strips; wv arrives
                    # while they run, wo (needed much later) after that
                    nc.sync.dma_start(wv_s[:], wv_ap)
                for kc in range(n_kc):
                    t = xbpool.tile([128, SUPER], BF16, tag="xb",
                                    name=f"xb_{sw}_{kc}")
                    nc.sync.dma_start(t[:], xb_ap[:, kc, ts0:ts0 + SUPER])
                    xbt.append(t)
                if sw == 0:
                    nc.sync.dma_start(wo_s[:], wo_ap)

                for win in range(n_win):
                    wsl = slice(win * WIN, (win + 1) * WIN)
                    toks = slice(ts0 + win * WIN, ts0 + (win + 1) * WIN)
                    q_ps = [ps_qk.tile([128, WIN], F32, tag=f"q{h}",
                                       name=f"q_ps{h}") for h in range(H_LOC)]
                    k_ps = [ps_qk.tile([128, WIN], F32, tag=f"k{h}",
                                       name=f"k_ps{h}") for h in range(H_LOC)]
                    v_ps = [ps_v.tile([128, C_LOC], F32, tag=f"v{j}", bufs=1,
                                      name=f"v_ps{j}") for j in range(sub)]
                    if QK_FP8:
                        for pair in range(n_pair):
                            st = (pair == 0)
                            sp = (pair == n_pair - 1)
                            rhs = x8t[pair][:, :, wsl]
                            for h in range(H_LOC):
                                hs = slice(h * 128, (h + 1) * 128)
                                nc.tensor.matmul(
                                    q_ps[h][:], wq_s[:, pair, :, hs], rhs,
                                    start=st, stop=sp,
                                    perf_mode=mybir.MatmulPerfMode.DoubleRow)
                                nc.tensor.matmul(
                                    k_ps[h][:], wk_s[:, pair, :, hs], rhs,
                                    start=st, stop=sp,
                                    perf_mode=mybir.MatmulPerfMode.DoubleRow)
                    else:
                        for kc in range(n_kc):
                            st = (kc == 0)
                            sp = (kc == n_kc - 1)
                            rhs = xbt[kc][:, wsl]
                            for h in range(H_LOC):
                                hs = slice(h * 128, (h + 1) * 128)
                                nc.tensor.matmul(q_ps[h][:], wq_s[:, kc, hs],
                                                 rhs, start=st, stop=sp)
                                nc.tensor.matmul(k_ps[h][:], wk_s[:, kc, hs],
                                                 rhs, start=st, stop=sp)
                    # V: x chunk stationary, wv moving; one matmul per
                    # (kc, token-subtile), PSUM-accumulated over kc
                    for kc in range(n_kc):
                        st = (kc == 0)
                        sp = (kc == n_kc - 1)
                        for j in range(sub):
                            nc.tensor.matmul(
                                v_ps[j][:],
                                xbt[kc][:, win * WIN + j * 128:
                                        win * WIN + (j + 1) * 128],
                                wv_s[:, kc, :], start=st, stop=sp)
                    for h in range(H_LOC):
                        nc.scalar.copy(qT_s[:, h, toks], q_ps[h][:])
                        nc.scalar.copy(kT_s[:, h, toks], k_ps[h][:])
                    base_vt = (sw * n_win + win) * sub
                    for j in range(sub):
                        nc.vector.tensor_copy(v_s[:, base_vt + j, :],
                                              v_ps[j][:])

        if dbg_mode == 2:
            # dump projections before attention runs
            nc.sync.dma_start(dq_ap, qT_s[:])
            nc.sync.dma_start(dk_ap, kT_s[:])
            nc.sync.dma_start(dv_ap, v_s[:])

        # ---- Stage 2: attention + interleaved out-projection ----------
        with nc.named_scope("attention"), ExitStack() as s2:
            ptpool = s2.enter_context(tc.tile_pool(name="ptpool", bufs=8))
            spool = s2.enter_context(tc.tile_pool(name="spool", bufs=4))
            ypool = s2.enter_context(tc.tile_pool(name="ypool", bufs=4))
            ps_at = s2.enter_context(
                tc.tile_pool(name="ps_at", bufs=1, space="PSUM"))

            out_pieces = []  # (bt, nw) out-projection work items
            y_stage = {}  # bt -> staging tile
            piece_ctr = [0]

            def emit_piece():
                if not out_pieces:
                    return False
                bt, nw = out_pieces.pop(0)
                piece_ctr[0] += 1
                rows = slice(bt * 128, (bt + 1) * 128)
                cols = slice(nw * WIN, (nw + 1) * WIN)
                if nw == 0:
                    y_stage[bt] = ypool.tile([128, Cc], BF16, tag="ysb",
                                             name=f"y{bt}")
                y_sb = y_stage[bt]
                y_ps = ps_at.tile([128, WIN], F32, tag="sty", bufs=4,
                                  name=f"y_ps{bt}_{nw}")
                for hc in range(H_LOC):
                    nc.tensor.matmul(y_ps[:], ot_s[:, hc, rows],
                                     wo_s[:, hc, cols],
                                     start=(hc == 0),
                                     stop=(hc == H_LOC - 1))
                nc.vector.tensor_copy(y_sb[:, cols], y_ps[:])
                if nw == n_nw - 1:
                    nc.sync.dma_start(y_ap[rows, :], y_sb[:])
                    del y_stage[bt]
                return True

            for b in range(Bb):
                for qw in range(n_qw):
                    qoff = b * Tt + qw * WIN
                    qsl = slice(qoff, qoff + WIN)
                    n_kt = sub * (qw + 1)
                    ot_ps = [ps_at.tile([128, WIN], F32, tag=f"ot{h}", bufs=1,
                                        name=f"ot_ps{h}") for h in range(H_LOC)]
                    s_ps = [ps_at.tile([1, WIN], F32, tag=f"s{h}", bufs=1,
                                       name=f"s_ps{h}") for h in range(H_LOC)]

                    def col_start(kt):
                        kt_rel = kt - qw * sub
                        return max(kt_rel, 0) * 128

                    def st_pair(kt):
                        koff = b * Tt + kt * 128
                        vs = col_start(kt)
                        ts = []
                        for h in range(H_LOC):
                            t = ps_at.tile([128, WIN], F32, tag="sty",
                                           bufs=4, name=f"st_ps{h}")
                            nc.tensor.matmul(
                                t[:, vs:], kT_s[:, h, koff:koff + 128],
                                qT_s[:, h, qoff + vs:qoff + WIN],
                                start=True, stop=True)
                            ts.append(t)
                        return ts

                    st_next = st_pair(0)
                    emit_piece()
                    emit_piece()
                    for kt in range(n_kt):
                        vs = col_start(kt)
                        st_cur = st_next
                        if kt + 1 < n_kt:
                            st_next = st_pair(kt + 1)
                        first = (kt == 0)
                        last = (kt == n_kt - 1)
                        vt = b * (Tt // 128) + kt
                        diag = (kt >= qw * sub)
                        pts = []
                        for h in range(H_LOC):
                            pt = ptpool.tile([128, WIN], BF16, tag="pt",
                                             name=f"pt{h}")
                            nc.scalar.activation(
                                pt[:, vs:], st_cur[h][:, vs:],
                                mybir.ActivationFunctionType.Exp,
                                scale=EXP_SCALE)
                            if diag:
                                # zero the strict upper triangle of the
                                # diagonal 128-block (global_k > global_q)
                                nc.gpsimd.affine_select(
                                    out=pt[:, vs:vs + 128],
                                    in_=pt[:, vs:vs + 128],
                                    compare_op=mybir.AluOpType.is_ge,
                                    fill=0.0, base=0,
                                    pattern=[[1, 128]],
                                    channel_multiplier=-1,
                                )
                            pts.append(pt)
                        for h in range(H_LOC):
                            nc.tensor.matmul(ot_ps[h][:, vs:],
                                             v_s[:, vt, h * 128:(h + 1) * 128],
                                             pts[h][:, vs:],
                                             start=first, stop=last)
                            nc.tensor.matmul(
                                s_ps[h][:, vs:], ones_col[:],
                                pts[h][:, vs:], start=first, stop=last)
                        # out-projection pieces of finished windows keep the
                        # PE fed while ACT works through the exp chain
                        emit_piece()
                        if len(out_pieces) > 12:
                            emit_piece()

                    # rowsum -> SBUF so the PSUM bank recycles quickly
                    s_sb = [spool.tile([1, WIN], F32, tag=f"ssb{h}",
                                       name=f"ssb{h}") for h in range(H_LOC)]
                    for h in range(H_LOC):
                        nc.scalar.copy(s_sb[h][:], s_ps[h][:])

                    # normalize immediately: DVE/gpsimd run this while the
                    # next window's STs and outproj pieces occupy the PE
                    for h in range(H_LOC):
                        srec = spool.tile([1, WIN], F32, tag="srec",
                                          name="srec")
                        nc.vector.reciprocal_approx_fast(srec[:], s_sb[h][:])
                        bc = spool.tile([128, WIN], F32, tag="bc", name="bc")
                        nc.gpsimd.partition_broadcast(bc[:], srec[:])
                        nc.vector.tensor_mul(ot_s[:, h, qsl], ot_ps[h][:],
                                             bc[:])
                    for bt in range(qw * sub, (qw + 1) * sub):
                        for nw in range(n_nw):
                            out_pieces.append((b * (Tt // 128) + bt, nw))

            with nc.named_scope("flush"):
                while emit_piece():
                    pass
                if dbg:
                    if dbg_mode == 1:
                        nc.sync.dma_start(dq_ap, qT_s[:])
                        nc.sync.dma_start(dk_ap, kT_s[:])
                        nc.sync.dma_start(dv_ap, v_s[:])
                    nc.sync.dma_start(dot_ap, ot_s[:])

    nc.compile()
    return nc


_PROGRAM = None


def _get_program():
    global _PROGRAM
    if _PROGRAM is None:
        _PROGRAM = build_program()
    return _PROGRAM


def make_in_maps(x, Wq, Wk, Wv, Wo):
    """Host-side sharding: build the per-core input dicts."""
    import ml_dtypes

    x = np.asarray(x, dtype=np.float32)
    Wq = np.asarray(Wq, dtype=np.float32)
    Wk = np.asarray(Wk, dtype=np.float32)
    Wv = np.asarray(Wv, dtype=np.float32)
    Wo = np.asarray(Wo, dtype=np.float32)
    Bb, Tt, Cc = x.shape
    BT = Bb * Tt
    n_kc = Cc // 128
    n_pair = n_kc // 2
    xT = np.ascontiguousarray(x.reshape(BT, Cc).T)  # [C, BT]

    xb = np.ascontiguousarray(
        xT.astype(ml_dtypes.bfloat16).reshape(n_kc, 128, BT)
        .transpose(1, 0, 2))
    if QK_FP8:
        x8 = np.ascontiguousarray(
            (xT * SX).astype(ml_dtypes.float8_e4m3fn)
            .reshape(n_pair, 2, 128, BT).transpose(2, 0, 1, 3))

    scale_q = 1.0 / math.sqrt(HEAD_DIM)
    in_maps = []
    for c in range(N_CORES):
        rows = slice(c * C_LOC, (c + 1) * C_LOC)
        WqT = Wq[rows, :].T * scale_q  # [C, C_LOC]
        WkT = Wk[rows, :].T
        WvT = Wv[rows, :].T
        WoT = Wo[:, rows].T  # [C_LOC, C]
        if QK_FP8:
            wq = np.ascontiguousarray(
                (WqT * SQ).astype(ml_dtypes.float8_e4m3fn)
                .reshape(n_pair, 2, 128, C_LOC).transpose(2, 0, 1, 3))
            wk = np.ascontiguousarray(
                (WkT * SK).astype(ml_dtypes.float8_e4m3fn)
                .reshape(n_pair, 2, 128, C_LOC).transpose(2, 0, 1, 3))
        else:
            wq = np.ascontiguousarray(
                WqT.astype(ml_dtypes.bfloat16).reshape(n_kc, 128, C_LOC)
                .transpose(1, 0, 2))
            wk = np.ascontiguousarray(
                WkT.astype(ml_dtypes.bfloat16).reshape(n_kc, 128, C_LOC)
                .transpose(1, 0, 2))
        wv = np.ascontiguousarray(
            WvT.astype(ml_dtypes.bfloat16).reshape(n_kc, 128, C_LOC)
            .transpose(1, 0, 2))
        wo = np.ascontiguousarray(
            WoT.astype(ml_dtypes.bfloat16).reshape(H_LOC, 128, Cc)
            .transpose(1, 0, 2))
        m = {"xb": xb, "wq": wq, "wk": wk, "wv": wv, "wo": wo}
        if QK_FP8:
            m["x8"] = x8
        in_maps.append(m)
    return in_maps


def kernel(x, Wq, Wk, Wv, Wo):
    from concourse.bass_utils import run_bass_kernel_spmd

    nc = _get_program()
    in_maps = make_in_maps(x, Wq, Wk, Wv, Wo)
    res = run_bass_kernel_spmd(nc, in_maps, list(range(N_CORES)))
    x = np.asarray(x)
    Bb, Tt, Cc = x.shape
    y = np.zeros((Bb * Tt, Cc), dtype=np.float32)
    for c in range(N_CORES):
        y += res.results[c]["y"].astype(np.float32)
    return y.reshape(Bb, Tt, Cc)
